# revision 4
# baseline (speedup 1.0000x reference)
"""Trainium2 Bass kernel for the dendritic template-gated FFN.

Math (per output feature h, token n; W=16 windows of K=64 features):
    s[n,h,w] = <x[n, w*64:(w+1)*64], t[h, w*64:(w+1)*64]>
    out[n,h] = sum_w softmax_w(s) * silu(s)
             = (sum_w e^{s_w} silu(s_w)) / (sum_w e^{s_w})

Kernel strategy (8 NeuronCores, data-parallel over tokens, 512 each):
  - PE: per-window matmuls s_w = x_w @ t_w^T with K=64 on partitions
    (both operands pre-transposed on-chip via PE-transpose).
  - ACT: e = exp(s), th = tanh(s/2)   (same table set -> one table load)
  - DVE: phi = (th + 1) * e  (= 2 e^s sigmoid(s));  p = s * phi (= 2 e^s silu(s))
  - PE: window-reduction via identity-matmul PSUM accumulation:
        den += (2I) @ e_w ; num += I @ p_w   => num/den == out exactly
  - DVE: r = 1/den (2-ULP approx), out = num * r
"""

import numpy as np
from contextlib import ExitStack

import concourse.bass as bass
import concourse.bacc as bacc
import concourse.mybir as mybir
import concourse.tile as tile
from concourse.bass_utils import run_bass_kernel_spmd

AF = mybir.ActivationFunctionType
ALU = mybir.AluOpType
DT = mybir.dt

N_TOTAL = 4096
IN_F = 1024
OUT_F = 2048
WIN = 64
NW = 16          # windows
N_CORES = 8
N_SH = N_TOTAL // N_CORES   # 512 tokens per core

# windows per PSUM group (3 banks of s at a time)
GROUPS = [(0, 1, 2), (3, 4, 5), (6, 7, 8), (9, 10, 11), (12, 13, 14), (15,)]


def build_program(n_tok=N_SH, mm_dt=DT.float32, red_dt=DT.float32):
    """Build the single-core Bass program (same NEFF runs SPMD on all cores)."""
    nc = bacc.Bacc(
        "TRN2",
        target_bir_lowering=False,
        debug=False,
        enable_asserts=False,
        num_devices=N_CORES,
    )
    x_d = nc.dram_tensor("x", [n_tok, IN_F], DT.float32, kind="ExternalInput").ap()
    t_d = nc.dram_tensor(
        "template_flat", [OUT_F, IN_F], DT.float32, kind="ExternalInput"
    ).ap()
    eye_d = nc.dram_tensor("eye12", [128, 256], DT.float32, kind="ExternalInput").ap()
    out_d = nc.dram_tensor("out", [n_tok, OUT_F], DT.float32, kind="ExternalOutput").ap()

    NT = n_tok // 128       # token tiles
    NJ = OUT_F // 512       # h chunks
    KB = IN_F // 128        # 128-wide k blocks (2 windows each)
    HB = OUT_F // 128       # 128-row h blocks of template

    def mmc(ap):
        return ap.bitcast(mm_dt) if mm_dt != DT.float32 else ap

    def redc(ap):
        return ap.bitcast(red_dt) if red_dt != DT.float32 else ap

    with ExitStack() as ctx:
        tc = ctx.enter_context(tile.TileContext(nc))

        const_pool = ctx.enter_context(tc.tile_pool(name="const", bufs=1))
        eye_t = const_pool.tile([128, 256], DT.float32, tag="eye")
        nc.sync.dma_start(eye_t[:], eye_d[:])
        eye1 = eye_t[:, 0:128]     # identity
        eye2 = eye_t[:, 128:256]   # 2 * identity

        persist = ctx.enter_context(tc.tile_pool(name="persist", bufs=1))
        xT = [persist.tile([128, n_tok], DT.float32, tag=f"xT{kb}", name=f"xT{kb}") for kb in range(KB)]
        tT = [persist.tile([128, OUT_F], DT.float32, tag=f"tT{kb}", name=f"tT{kb}") for kb in range(KB)]

        # ---- prologue: load x and template, transpose to K-major via PE ----
        # one DMA per staging tile (keeps per-instruction sync-wait count low)
        with tc.tile_pool(name="stage", bufs=1) as stage, \
             tc.tile_pool(name="tpsum", bufs=2, space="PSUM") as tpsum:
            x_nm = []
            for i in range(NT):
                xt_ = stage.tile([128, IN_F], DT.float32, tag=f"xnm{i}",
                                 name=f"xnm{i}")
                nc.sync.dma_start(
                    xt_[:], x_d.rearrange("(i p) k -> p i k", p=128)[:, i, :]
                )
                x_nm.append(xt_)
            t_nm = []
            for hb in range(HB):
                tt_ = stage.tile([128, IN_F], DT.float32, tag=f"tnm{hb}",
                                 name=f"tnm{hb}")
                nc.sync.dma_start(
                    tt_[:], t_d.rearrange("(h p) k -> p h k", p=128)[:, hb, :]
                )
                t_nm.append(tt_)

            for kb in range(KB):
                ps = tpsum.tile([128, NT * 128], DT.float32, tag="xps")
                for i in range(NT):
                    nc.tensor.transpose(
                        ps[:, i * 128:(i + 1) * 128],
                        x_nm[i][:, kb * 128:(kb + 1) * 128],
                        eye1,
                    )
                nc.vector.tensor_copy(xT[kb][:], ps[:])

            for kb in range(KB):
                for hh in range(0, HB, 8):  # 8 h-blocks -> [128, 1024] psum (2 banks)
                    ps = tpsum.tile([128, 1024], DT.float32, tag="tps")
                    for q in range(8):
                        hb = hh + q
                        nc.tensor.transpose(
                            ps[:, q * 128:(q + 1) * 128],
                            t_nm[hb][:, kb * 128:(kb + 1) * 128],
                            eye1,
                        )
                    nc.vector.tensor_copy(
                        tT[kb][:, hh * 128:(hh + 8) * 128], ps[:]
                    )

        # ---- main loop ----
        spool = ctx.enter_context(tc.tile_pool(name="spsum", bufs=2, space="PSUM"))
        dnpool = ctx.enter_context(tc.tile_pool(name="dnpsum", bufs=1, space="PSUM"))
        e_pool = ctx.enter_context(tc.tile_pool(name="epool", bufs=2))
        th_pool = ctx.enter_context(tc.tile_pool(name="thpool", bufs=2))
        phi_pool = ctx.enter_context(tc.tile_pool(name="phipool", bufs=2))
        p_pool = ctx.enter_context(tc.tile_pool(name="ppool", bufs=2))
        tail_pool = ctx.enter_context(tc.tile_pool(name="tail", bufs=2))

        for i in range(NT):
            for j in range(NJ):
                dn = dnpool.tile([128, 1024], DT.float32, tag="dn")
                den = dn[:, 0:512]
                num = dn[:, 512:1024]
                for grp in GROUPS:
                    glen = len(grp)
                    fd = glen * 512
                    st = spool.tile([128, 1536], DT.float32, tag="s")
                    for widx, w in enumerate(grp):
                        base = (w % 2) * 64
                        lhsT = xT[w // 2][base:base + 64, i * 128:(i + 1) * 128]
                        rhs = tT[w // 2][base:base + 64, j * 512:(j + 1) * 512]
                        nc.tensor.matmul(
                            st[:, widx * 512:(widx + 1) * 512],
                            mmc(lhsT), mmc(rhs),
                            start=True, stop=True, skip_group_check=True,
                        )
                    e_t = e_pool.tile([128, 1536], DT.float32, tag="e")
                    nc.scalar.activation(e_t[:, :fd], st[:, :fd], AF.Exp)
                    th_t = th_pool.tile([128, 1536], DT.float32, tag="th")
                    nc.scalar.activation(th_t[:, :fd], st[:, :fd], AF.Tanh, scale=0.5)
                    phi_t = phi_pool.tile([128, 1536], DT.float32, tag="phi")
                    nc.vector.scalar_tensor_tensor(
                        phi_t[:, :fd], th_t[:, :fd], 1.0, e_t[:, :fd],
                        ALU.add, ALU.mult,
                    )
                    p_t = p_pool.tile([128, 1536], DT.float32, tag="p")
                    nc.vector.tensor_tensor(
                        p_t[:, :fd], st[:, :fd], phi_t[:, :fd], ALU.mult
                    )
                    for widx, w in enumerate(grp):
                        sl = slice(widx * 512, (widx + 1) * 512)
                        nc.tensor.matmul(
                            den, redc(eye2), redc(e_t[:, sl]),
                            start=(w == 0), stop=(w == NW - 1),
                            skip_group_check=True,
                        )
                        nc.tensor.matmul(
                            num, redc(eye1), redc(p_t[:, sl]),
                            start=(w == 0), stop=(w == NW - 1),
                            skip_group_check=True,
                        )
                r_t = tail_pool.tile([128, 512], DT.float32, tag="r")
                sc_t = tail_pool.tile([128, 512], DT.float32, tag="sc")
                nc.vector.reciprocal_approx_accurate(r_t[:], den, scratch=sc_t[:])
                o_t = tail_pool.tile([128, 512], DT.float32, tag="o")
                nc.vector.tensor_tensor(o_t[:], num, r_t[:], ALU.mult)
                nc.sync.dma_start(
                    out_d[i * 128:(i + 1) * 128, j * 512:(j + 1) * 512], o_t[:]
                )

    nc.compile()
    return nc


_EYE = None
_PROG = None


def _eye_input():
    global _EYE
    if _EYE is None:
        e = np.eye(128, dtype=np.float32)
        _EYE = np.concatenate([e, 2.0 * e], axis=1)
    return _EYE


def kernel(x: np.ndarray, template_flat: np.ndarray) -> np.ndarray:
    global _PROG
    x = np.ascontiguousarray(x, dtype=np.float32)
    template_flat = np.ascontiguousarray(template_flat, dtype=np.float32)
    assert x.shape == (N_TOTAL, IN_F) and template_flat.shape == (OUT_F, IN_F)
    if _PROG is None:
        _PROG = build_program()
    eye = _eye_input()
    in_maps = [
        {
            "x": x[c * N_SH:(c + 1) * N_SH],
            "template_flat": template_flat,
            "eye12": eye,
        }
        for c in range(N_CORES)
    ]
    res = run_bass_kernel_spmd(_PROG, in_maps, core_ids=list(range(N_CORES)))
    return np.concatenate([r["out"] for r in res.results], axis=0)
